# revision 36
# baseline (speedup 1.0000x reference)
"""MemTransformerLM (Transformer-XL) forward pass on 8 TRN2 NeuronCores.

Sharding: core c handles batch b = c//2 and tensor-parallel half h = c%2
(heads 8h..8h+8 of 16; FFN inner columns 2048h..2048h+2048 of 4096).
Pairwise AllReduce (cores 2b, 2b+1) after the attention output projection and
after FFN. Vocab for the final logsumexp is split 16000 per core in the pair;
host combines per-tile (max, sumexp) partials and computes the NLL.

All matmuls run in bf16 with fp32 PSUM accumulation; the residual stream,
layernorm statistics, and softmax denominators stay fp32.

Attention works on TRANSPOSED scores (scoreT[j, i], keys on partitions) so
the softmax'd probs feed the PV matmul straight from SBUF with no transpose:
  AC^T:  matmul(lhsT=kT[64d, j-chunk], rhs=qbwT[64d, i])       -> [j, i] psum
  BD^T:  added into the same psum via an identity matmul from SBUF.
  exp:   ACT psum -> SBUF bf16 probT (no max-subtraction: |s*scale| < ~8).
  PV:    matmul(lhsT=vv[j, 65-wide head block], rhs=probT) accumulated over
         j-chunks; the 65th lhsT column is all-ones, so psum row 64 is the
         softmax denominator for free. A reciprocal + partition-broadcast +
         multiply normalizes on the way out to pvT_all.

rel_shift+mask trick: BD[i, j] = pre[i, r=j-i+qlen-1] where pre[i, r] =
(q_i+br)@rk_r. pre rows are written to a DRAM buffer with row stride W=1536
(cols 0..1023 = pre, cols 1024..1535 pre-filled with -1e30 once). Reading
flat[511 + 1535*i + j] gives the shifted matrix; masked positions (j > i+512
<=> r > 1023) land exactly in the -1e30 pad, so no mask op is needed at all.
One dma_start_transpose with that strided source AP yields bdT[j, i] directly.
"""

import numpy as np
import ml_dtypes

import concourse.bass as bass
import concourse.mybir as mybir
import concourse.tile as tile
from concourse import bacc
from concourse.bass_utils import run_bass_kernel_spmd

# Model dims (hardcoded per problem spec)
L = 6
D_MODEL = 1024
D_HEAD = 64
D_INNER = 4096
BSZ = 4
QLEN = 512
MLEN = 512
KLEN = MLEN + QLEN
VOCAB = 32000
SCALE = 1.0 / (D_HEAD ** 0.5)
EPS = 1e-5

NCORES = 8
NDH = 512          # nd per core (8 heads x 64)
DIH = 2048         # ffn inner per core
VSH = VOCAB // 2   # vocab per core (split across the pair)
VT = 500           # vocab tile width
NVT = VSH // VT    # 32
PREW = KLEN + 512  # pre-buffer row width (cols KLEN.. = -1e30 mask pad)
SCR_BUFS = 3       # dram pool slots for the pre buffer

DT = mybir.dt.float32
BF = mybir.dt.bfloat16
F32 = np.float32
BF16 = ml_dtypes.bfloat16

PAIRS = [[0, 1], [2, 3], [4, 5], [6, 7]]

DEBUG_DUMP = False  # add bdT/probT/pv debug outputs for layer 0, head 0

_CACHE: dict = {}


def _build():
    nc = bacc.Bacc("TRN2", target_bir_lowering=False, debug=False, num_devices=NCORES)

    # ---- I/O ----
    x0_in = nc.dram_tensor("x0", [128, 4, D_MODEL], DT, kind="ExternalInput")
    memT_in = nc.dram_tensor("memT", [L, 128, 8, MLEN], BF, kind="ExternalInput")
    wq_in = nc.dram_tensor("wq", [L, 128, 8, NDH], BF, kind="ExternalInput")
    wk_in = nc.dram_tensor("wk", [L, 128, 8, NDH], BF, kind="ExternalInput")
    wv_in = nc.dram_tensor("wv", [L, 128, 8, NDH], BF, kind="ExternalInput")
    rkT_in = nc.dram_tensor("rkT", [L, 4, 128, KLEN], BF, kind="ExternalInput")
    wo_in = nc.dram_tensor("wo", [L, 128, 4, D_MODEL], BF, kind="ExternalInput")
    w1_in = nc.dram_tensor("w1", [L, 128, 8, DIH], BF, kind="ExternalInput")
    w2_in = nc.dram_tensor("w2", [L, 128, 16, D_MODEL], BF, kind="ExternalInput")
    b1_in = nc.dram_tensor("b1", [L, 128, 16], DT, kind="ExternalInput")
    b2_in = nc.dram_tensor("b2", [L, D_MODEL], DT, kind="ExternalInput")
    g1_in = nc.dram_tensor("g1", [L, D_MODEL], DT, kind="ExternalInput")
    bg1_in = nc.dram_tensor("bg1", [L, D_MODEL], DT, kind="ExternalInput")
    g2_in = nc.dram_tensor("g2", [L, D_MODEL], DT, kind="ExternalInput")
    bg2_in = nc.dram_tensor("bg2", [L, D_MODEL], DT, kind="ExternalInput")
    bw_in = nc.dram_tensor("bw", [128, 4], DT, kind="ExternalInput")
    br_in = nc.dram_tensor("br", [128, 4], DT, kind="ExternalInput")
    embT_in = nc.dram_tensor("embT", [NVT, 128, 8, VT], BF, kind="ExternalInput")

    xout = nc.dram_tensor("xout", [QLEN, D_MODEL], DT, kind="ExternalOutput")
    lmax_out = nc.dram_tensor("lmax", [128, 4, NVT], DT, kind="ExternalOutput")
    lsum_out = nc.dram_tensor("lsum", [128, 4, NVT], DT, kind="ExternalOutput")
    if DEBUG_DUMP:
        dbg_bdT = nc.dram_tensor("dbg_bdT", [128, 8, QLEN], BF, kind="ExternalOutput")
        dbg_prT = nc.dram_tensor("dbg_prT", [128, 8, QLEN], BF, kind="ExternalOutput")
        dbg_pv = nc.dram_tensor("dbg_pv", [64, QLEN], BF, kind="ExternalOutput")
        dbg_den = nc.dram_tensor("dbg_den", [1, QLEN], DT, kind="ExternalOutput")
        dbg_recb = nc.dram_tensor("dbg_recb", [64, QLEN], DT, kind="ExternalOutput")
        dbg_vv = nc.dram_tensor("dbg_vv", [128, 65], BF, kind="ExternalOutput")

    with tile.TileContext(nc) as tc:
        with (
            tc.tile_pool(name="const", bufs=1) as constp,
            tc.tile_pool(name="res", bufs=1) as resp,
            tc.tile_pool(name="wts", bufs=1) as wtp,
            tc.tile_pool(name="act", bufs=1) as actp,
            tc.tile_pool(name="xt2", bufs=2) as xtp,
            tc.tile_pool(name="ncc", bufs=2) as nccp,
            tc.tile_pool(name="arp", bufs=1) as arp,
            tc.tile_pool(name="tr", bufs=2) as trp,
            tc.tile_pool(name="bdp", bufs=2) as bdp,
            tc.tile_pool(name="pr2", bufs=1) as prp,
            tc.tile_pool(name="rcp", bufs=1) as rcp,
            tc.tile_pool(name="small", bufs=2) as smp,
            tc.tile_pool(name="ps_sc", bufs=4, space="PSUM") as psS,
            tc.tile_pool(name="ps_proj", bufs=2, space="PSUM") as psP,
            tc.tile_pool(name="ps_pv", bufs=2, space="PSUM") as psV,
            tc.tile_pool(name="dram", bufs=3, space="DRAM") as dramp,
        ):
            psO = psP  # out-proj / FFN2 share the projection psum slots
            bw_t = constp.tile([128, 4], DT)
            br_t = constp.tile([128, 4], DT)
            nc.sync.dma_start(bw_t[:], bw_in[:])
            nc.sync.dma_start(br_t[:], br_in[:])

            # 128x128 bf16 identity (for the BD += identity @ bdT psum adds)
            ident = constp.tile([128, 128], BF)
            nc.vector.memset(ident[:], 1.0)
            nc.gpsimd.affine_select(
                out=ident[:], in_=ident[:], pattern=[[-1, 128]],
                compare_op=mybir.AluOpType.is_equal, fill=0.0,
                base=0, channel_multiplier=1,
            )

            # -1e30 pad source for the pre-buffer mask columns
            padsb = constp.tile([128, 512], BF)
            nc.vector.memset(padsb[:], -1e30)

            # residual stream, fp32, one tile per 128-query chunk so the four
            # AG-chunk -> residual -> layernorm chains run independently
            x_res = [resp.tile([128, D_MODEL], DT, name=f"xres{qc}",
                               tag=f"xres{qc}")
                     for qc in range(4)]
            for qc in range(4):
                nc.sync.dma_start(x_res[qc][:], x0_in[:, qc, :])
            lmax_sb = resp.tile([128, 4, NVT], DT)
            lsum_sb = resp.tile([128, 4, NVT], DT)

            # persistent pre buffers (explicit round-robin across heads);
            # mask pad columns KLEN..PREW filled with -1e30 exactly once
            scr_bufs = []
            for i in range(SCR_BUFS):
                scr = dramp.tile([QLEN * PREW], BF, tag=f"bdsc{i}")
                scr2d = scr.rearrange("(q w) -> q w", w=PREW)
                for qc in range(4):
                    nc.sync.dma_start(
                        scr2d[128 * qc : 128 * qc + 128, KLEN:PREW], padsb[:]
                    )
                scr_bufs.append((scr, scr2d))
            scr_rr = [0]  # round-robin cursor

            # persistent PV lhsT tile; ones half-columns set once
            vv_all = actp.tile([128, 8, 8 * 2 * D_HEAD], BF, tag="vvf")
            vv_ones = vv_all.rearrange("p k (h w) -> p k h w", w=2 * D_HEAD)
            nc.vector.memset(vv_ones[:, :, :, D_HEAD : 2 * D_HEAD], 1.0)

            def transpose_x():
                """Transpose x_res into [128, 8(dc), QLEN] bf16 via cast-DMA to DRAM
                plus two XBAR transpose-DMAs (chunk layout: row = 128*dc + p)."""
                xsc = dramp.tile([QLEN, D_MODEL], BF, tag="xsc")
                x2d = xsc.rearrange("(c p) d -> p c d", p=128)
                for qc in range(4):
                    nc.gpsimd.dma_start(x2d[:, qc, :], x_res[qc][:])
                dest = xtp.tile([128, 8, QLEN], BF, tag="xt")
                nc.sync.dma_start_transpose(dest[:, 0:4, :], xsc[:, 0 : D_MODEL // 2])
                nc.sync.dma_start_transpose(dest[:, 4:8, :], xsc[:, D_MODEL // 2 :])
                return dest

            for l in range(L):
                # ---- weight loads (wq aliases wo's slot: disjoint lifetimes) ----
                wq_t = wtp.tile([128, 8, NDH], BF, tag="wqo")
                wk_t = wtp.tile([128, 8, NDH], BF, tag="wk")
                wv_t = wtp.tile([128, 8, NDH], BF, tag="wv")
                w1_t = wtp.tile([128, 8, DIH], BF, tag="wff")
                nc.sync.dma_start(wq_t[:], wq_in[l])
                nc.sync.dma_start(wk_t[:], wk_in[l])
                nc.sync.dma_start(wv_t[:], wv_in[l])
                nc.sync.dma_start(w1_t[:], w1_in[l])
                b1_t = wtp.tile([128, 16], DT, tag="bb")
                nc.sync.dma_start(b1_t[:], b1_in[l])

                memT_t = actp.tile([128, 8, MLEN], BF, tag="memT")
                nc.sync.dma_start(memT_t[:], memT_in[l])
                xT_t = transpose_x()

                # ---- attention ----
                # v (all heads) as PV lhsT blocks of 128: cols [128h,128h+64)
                # = v, cols [128h+64,128h+128) = 1.0, so PV psum rows 64:128
                # hold the softmax denominator replicated 64x (lane-aligned
                # for the reciprocal + normalize multiply).
                vv4 = vv_all.rearrange("p k (h w) -> p k h w", w=2 * D_HEAD)
                for kc in range(8):
                    vps = psP.tile([128, QLEN], DT, tag="proj")
                    src = memT_t if kc < 4 else xT_t
                    ksl = slice(128 * (kc % 4), 128 * (kc % 4) + 128)
                    for dc in range(8):
                        nc.tensor.matmul(
                            vps[:], src[:, dc, ksl], wv_t[:, dc, :],
                            start=(dc == 0), stop=(dc == 7),
                        )
                    vps8 = vps.rearrange("p (h w) -> p h w", w=D_HEAD)
                    if kc % 2 == 0:
                        nc.scalar.copy(vv4[:, kc, :, 0:D_HEAD], vps8[:])
                    else:
                        nc.vector.tensor_copy(vv4[:, kc, :, 0:D_HEAD], vps8[:])
                vv = vv_all
                pvT_all = actp.tile([128, 4, QLEN], BF, tag="pvT")

                def ncc_proj(ncc):
                    nsl = slice(128 * ncc, 128 * ncc + 128)
                    # q^T (+bw / +br) for this ncc chunk
                    qps = psP.tile([128, QLEN], DT, tag="proj")
                    for dc in range(8):
                        nc.tensor.matmul(
                            qps[:], wq_t[:, dc, nsl], xT_t[:, dc, :],
                            start=(dc == 0), stop=(dc == 7),
                        )
                    qbwT = nccp.tile([128, QLEN], BF, tag="qbw")
                    qbrT = nccp.tile([128, QLEN], BF, tag="qbr")
                    nc.scalar.add(qbwT[:], qps[:], bw_t[:, ncc : ncc + 1])
                    nc.scalar.add(qbrT[:], qps[:], br_t[:, ncc : ncc + 1])
                    # k^T for this ncc chunk
                    kT = nccp.tile([128, KLEN], BF, tag="kT")
                    for kh in range(2):
                        kps = psP.tile([128, QLEN], DT, tag="proj")
                        src = memT_t if kh == 0 else xT_t
                        for dc in range(8):
                            nc.tensor.matmul(
                                kps[:], wk_t[:, dc, nsl], src[:, dc, :],
                                start=(dc == 0), stop=(dc == 7),
                            )
                        nc.vector.tensor_copy(kT[:, 512 * kh : 512 * kh + 512], kps[:])
                    # rk^T for this ncc chunk (host-computed rk = pos_emb @ Wr)
                    rkT = nccp.tile([128, KLEN], BF, tag="rkT")
                    nc.sync.dma_start(rkT[:], rkT_in[l, ncc])
                    return qbwT, qbrT, kT, rkT

                def pre_phase(ncc, hh, qbwT, qbrT, kT, rkT):
                    base = 64 * hh
                    # pre buffer: rows of width W=1536; cols 1024: hold -1e30
                    scr, scr2d = scr_bufs[scr_rr[0] % SCR_BUFS]
                    scr_rr[0] += 1
                    # pre = (q+br)^T-chunk @ rkT, written to DRAM
                    for qc in range(4):
                        pre_sb = trp.tile([128, KLEN], BF, tag="pre_sb")
                        for kh in range(2):
                            pre = psP.tile([128, 512], DT, tag="proj")
                            nc.tensor.matmul(
                                pre[:],
                                qbrT[base : base + 64, 128 * qc : 128 * qc + 128],
                                rkT[base : base + 64, 512 * kh : 512 * kh + 512],
                                start=True, stop=True,
                            )
                            if kh == 0:
                                nc.vector.tensor_copy(
                                    pre_sb[:, 512 * kh : 512 * kh + 512], pre[:]
                                )
                            else:
                                nc.scalar.copy(
                                    pre_sb[:, 512 * kh : 512 * kh + 512], pre[:]
                                )
                        nc.sync.dma_start(
                            scr2d[128 * qc : 128 * qc + 128, 0:KLEN], pre_sb[:]
                        )
                    # shifted + transposed reload: bdT[128p, kc, i] = BD^T[j, i]
                    # with j = 128*kc + p; masked j land in the -1e30 pad.
                    bdT = bdp.tile([128, 8, QLEN], BF, tag="bdT")
                    shifted = bass.AP(
                        scr.tensor,
                        scr.offset + (QLEN - 1),
                        [[PREW - 1, QLEN], [1, KLEN]],
                    )
                    nc.scalar.dma_start_transpose(bdT[:], shifted)
                    return bdT

                def score_phase(ncc, hh, qbwT, kT, bdT):
                    base = 64 * hh
                    h2 = 2 * ncc + hh
                    # scores (transposed) + exp, per 128-key chunk
                    probT = prp.tile([128, 8, QLEN], BF, tag=f"probT{hh}")
                    for kc in range(8):
                        sc = psS.tile([128, 512], DT, tag="sc")
                        nc.tensor.matmul(
                            sc[:], ident[:], bdT[:, kc, :],
                            start=True, stop=False,
                        )
                        nc.tensor.matmul(
                            sc[:],
                            kT[base : base + 64, 128 * kc : 128 * kc + 128],
                            qbwT[base : base + 64, :],
                            start=False, stop=True,
                        )
                        nc.scalar.activation(
                            probT[:, kc, :], sc[:],
                            mybir.ActivationFunctionType.Exp, scale=SCALE,
                        )
                    # PV; psum rows 64:128 = denominator (replicated 64x)
                    pv = psV.tile([128, QLEN], DT, tag="pv")
                    for kc in range(8):
                        nc.tensor.matmul(
                            pv[:],
                            vv[:, kc, 128 * h2 : 128 * h2 + 128],
                            probT[:, kc, :],
                            start=(kc == 0), stop=(kc == 7),
                        )
                    rec = rcp.tile([128, QLEN], DT, tag="rec")
                    nc.vector.reciprocal(rec[64:128, :], pv[64:128, :])
                    nc.vector.tensor_tensor(
                        pvT_all[base : base + 64, ncc, :],
                        pv[0:64, :], rec[64:128, :], mybir.AluOpType.mult,
                    )
                    if DEBUG_DUMP and l == 0 and ncc == 0 and hh == 0:
                        nc.sync.dma_start(dbg_bdT[:], bdT[:])
                        nc.sync.dma_start(dbg_prT[:], probT[:])
                        nc.sync.dma_start(dbg_den[:], rec[64:65, :])
                        nc.sync.dma_start(dbg_recb[:], rec[64:128, :])
                        nc.sync.dma_start(dbg_vv[:], vv[:, 0, 0:65])
                        nc.sync.dma_start(
                            dbg_pv[:], pvT_all[base : base + 64, ncc, :]
                        )

                # software-pipelined head loop: head i+1's pre matmuls fill
                # head i's DMA round-trip (pre write + shifted transpose read)
                nccs = {}
                prev = None
                for idx in range(8):
                    ncc, hh = divmod(idx, 2)
                    if hh == 0:
                        nccs[ncc] = ncc_proj(ncc)
                    qbwT, qbrT, kT, rkT = nccs[ncc]
                    bdT = pre_phase(ncc, hh, qbwT, qbrT, kT, rkT)
                    if prev is not None:
                        score_phase(*prev)
                    prev = (ncc, hh, qbwT, kT, bdT)
                score_phase(*prev)

                # layernorm params (bf16, broadcast to all partitions);
                # parked in a bdT slot (dead once attention scores are done)
                lnb = bdp.tile([128, 8, QLEN], BF, tag="bdT")
                lnb = lnb.rearrange("p h q -> p (h q)")[:, 0 : 4 * D_MODEL]
                lnb = lnb.rearrange("p (i d) -> p i d", d=D_MODEL)
                for i, src in enumerate((g1_in, bg1_in, g2_in, bg2_in)):
                    lnrow = smp.tile([1, D_MODEL], BF, tag="lnrow")
                    nc.gpsimd.dma_start(lnrow[:], src[l : l + 1, :])
                    nc.gpsimd.partition_broadcast(lnb[:, i, :], lnrow[:])

                def layer_norm_qc(goff, qc):
                    xr = x_res[qc][:]
                    # one packed stats tile per chain:
                    # [0:12]=bn_stats (2 groups x 6), [12]=mu, [13]=var,
                    # [14]=std, [15]=rstd
                    st = smp.tile([128, 16], DT, tag=f"lnst{qc}")
                    nc.vector.bn_stats(
                        st[:, 0:12].rearrange("p (n s) -> p n s", s=6),
                        x_res[qc].rearrange("p (n s) -> p n s", s=512),
                    )
                    nc.vector.bn_aggr(
                        st[:, 12:14], st[:, 0:12].rearrange("p (n s) -> p n s", s=6)
                    )
                    nc.vector.tensor_scalar_add(st[:, 14:15], st[:, 13:14], EPS)
                    nc.scalar.sqrt(st[:, 14:15], st[:, 14:15])
                    nc.vector.reciprocal(st[:, 15:16], st[:, 14:15])
                    # x = (x - mu) * rstd, one fused pass
                    nc.vector.tensor_scalar(
                        xr, xr, st[:, 12:13], st[:, 15:16],
                        mybir.AluOpType.subtract, mybir.AluOpType.mult,
                    )
                    nc.vector.tensor_tensor(
                        xr, xr, lnb[:, 2 * goff, :], mybir.AluOpType.mult
                    )
                    nc.vector.tensor_tensor(
                        xr, xr, lnb[:, 2 * goff + 1, :], mybir.AluOpType.add
                    )

                # ---- attention out projection + chunked pairwise AllGather;
                # residual add + ln1 pipelined per 128-query chunk ----
                wo_t = wtp.tile([128, 4, D_MODEL], BF, tag="wqo")
                nc.sync.dma_start(wo_t[:], wo_in[l])
                ar_in = dramp.tile([QLEN, D_MODEL], BF, tag="arin")
                ar_out = dramp.tile([4, 2, 128, D_MODEL], BF, tag="arout")
                asb = arp.tile([128, 4, D_MODEL], BF, tag="ar_sb")
                for qc in range(4):
                    for kh in range(2):
                        ops = psO.tile([128, 512], DT, tag="proj")
                        for ncc in range(4):
                            nc.tensor.matmul(
                                ops[:],
                                pvT_all[:, ncc, 128 * qc : 128 * qc + 128],
                                wo_t[:, ncc, 512 * kh : 512 * kh + 512],
                                start=(ncc == 0), stop=(ncc == 3),
                            )
                        nc.vector.tensor_copy(
                            asb[:, qc, 512 * kh : 512 * kh + 512], ops[:]
                        )
                    nc.sync.dma_start(
                        ar_in[128 * qc : 128 * qc + 128, :], asb[:, qc, :]
                    )
                    nc.gpsimd.collective_compute(
                        "AllGather", mybir.AluOpType.bypass,
                        replica_groups=PAIRS,
                        ins=[ar_in[128 * qc : 128 * qc + 128, :].opt()],
                        outs=[ar_out[qc].opt()],
                    )
                for qc in range(4):
                    # both pair partials (own + partner) summed into asb chunk
                    nc.sync.dma_start(asb[:, qc, :], ar_out[qc, 0])
                    nc.gpsimd.dma_start(
                        asb[:, qc, :], ar_out[qc, 1], accum_op=mybir.AluOpType.add
                    )
                    nc.vector.tensor_tensor(
                        x_res[qc][:], x_res[qc][:], asb[:, qc, :],
                        mybir.AluOpType.add,
                    )
                    layer_norm_qc(0, qc)

                # ---- FFN ----
                xfT = transpose_x()
                hT = actp.tile([128, 16, QLEN], BF, tag="hT")
                for ic in range(16):
                    ps = psP.tile([128, QLEN], DT, tag="proj")
                    for dc in range(8):
                        nc.tensor.matmul(
                            ps[:], w1_t[:, dc, 128 * ic : 128 * ic + 128],
                            xfT[:, dc, :],
                            start=(dc == 0), stop=(dc == 7),
                        )
                    nc.scalar.activation(
                        hT[:, ic, :], ps[:], mybir.ActivationFunctionType.Relu,
                        bias=b1_t[:, ic : ic + 1],
                    )
                w2_t = wtp.tile([128, 16, D_MODEL], BF, tag="wff")
                nc.sync.dma_start(w2_t[:], w2_in[l])
                # b2/2 broadcast (host halves it; both pair partials carry it)
                b2b = wtp.tile([128, D_MODEL], BF, tag="bb")
                b2row = smp.tile([1, D_MODEL], BF, tag="b2row")
                nc.gpsimd.dma_start(b2row[:], b2_in[l : l + 1, :])
                nc.gpsimd.partition_broadcast(b2b[:], b2row[:])
                ar_in2 = dramp.tile([QLEN, D_MODEL], BF, tag="arin")
                ar_out2 = dramp.tile([4, 2, 128, D_MODEL], BF, tag="arout")
                asb2 = arp.tile([128, 4, D_MODEL], BF, tag="ar_sb")
                for qc in range(4):
                    for kh in range(2):
                        ops = psO.tile([128, 512], DT, tag="proj")
                        for ic in range(16):
                            nc.tensor.matmul(
                                ops[:],
                                hT[:, ic, 128 * qc : 128 * qc + 128],
                                w2_t[:, ic, 512 * kh : 512 * kh + 512],
                                start=(ic == 0), stop=(ic == 15),
                            )
                        nc.vector.tensor_tensor(
                            asb2[:, qc, 512 * kh : 512 * kh + 512], ops[:],
                            b2b[:, 512 * kh : 512 * kh + 512],
                            mybir.AluOpType.add,
                        )
                    nc.sync.dma_start(
                        ar_in2[128 * qc : 128 * qc + 128, :], asb2[:, qc, :]
                    )
                    nc.gpsimd.collective_compute(
                        "AllGather", mybir.AluOpType.bypass,
                        replica_groups=PAIRS,
                        ins=[ar_in2[128 * qc : 128 * qc + 128, :].opt()],
                        outs=[ar_out2[qc].opt()],
                    )
                for qc in range(4):
                    nc.sync.dma_start(asb2[:, qc, :], ar_out2[qc, 0])
                    nc.gpsimd.dma_start(
                        asb2[:, qc, :], ar_out2[qc, 1],
                        accum_op=mybir.AluOpType.add,
                    )
                    nc.vector.tensor_tensor(
                        x_res[qc][:], x_res[qc][:], asb2[:, qc, :],
                        mybir.AluOpType.add,
                    )
                    layer_norm_qc(1, qc)

            # ---- final hidden out + unembed partials ----
            # |logits| < ~10 here, so exp needs no max-subtraction; lmax
            # output stays 0 and the host combine handles it unchanged.
            xo4 = xout.rearrange("(c p) d -> p c d", p=128)
            for qc in range(4):
                nc.sync.dma_start(xo4[:, qc, :], x_res[qc][:])
            nc.vector.memset(lmax_sb[:], 0.0)
            uf = transpose_x()
            for vt in range(NVT):
                # double-buffer embT tiles by ping-ponging two dead weight slots
                et = wtp.tile([128, 8, VT], BF, tag=("wk" if vt % 2 == 0 else "wv"))
                nc.sync.dma_start(et[:], embT_in[vt])
                for qc in range(4):
                    lps = psP.tile([128, QLEN], DT, tag="proj")
                    for dc in range(8):
                        nc.tensor.matmul(
                            lps[:, 0:VT],
                            uf[:, dc, 128 * qc : 128 * qc + 128],
                            et[:, dc, :],
                            start=(dc == 0), stop=(dc == 7),
                        )
                    lsc = trp.tile([128, VT], BF, tag="lsc")
                    nc.scalar.activation(
                        lsc[:], lps[:, 0:VT], mybir.ActivationFunctionType.Exp,
                        accum_out=lsum_sb[:, qc, vt : vt + 1],
                    )
            nc.sync.dma_start(lmax_out[:], lmax_sb[:])
            nc.sync.dma_start(lsum_out[:], lsum_sb[:])

    nc.compile()
    return nc


def _get_nc():
    if "nc" not in _CACHE:
        _CACHE["nc"] = _build()
    return _CACHE["nc"]


def _make_pos():
    pos_seq = np.arange(KLEN - 1, -1, -1, dtype=F32)
    inv_freq = 1.0 / (10000.0 ** (np.arange(0, D_MODEL, 2, dtype=F32) / D_MODEL))
    sin_inp = np.outer(pos_seq, inv_freq).astype(F32)
    return np.concatenate([np.sin(sin_inp), np.cos(sin_inp)], -1).astype(F32)


def _prep_inputs(data, memory, emb, Wq, Wkv, Wr, Wo, ffW1, ffb1, ffW2, ffb2,
                 ln1_g, ln1_b, ln2_g, ln2_b, bias_w, bias_r):
    pos = _make_pos()                                  # [KLEN, D_MODEL]
    rk = np.einsum("kd,ldn->lkn", pos, Wr.astype(F32))  # [L, KLEN, 2*NDH]
    embT = np.ascontiguousarray(emb.T).astype(BF16)    # [D_MODEL, VOCAB]
    bwf = bias_w.reshape(-1).astype(F32)
    brf = bias_r.reshape(-1).astype(F32)

    def chunk(w, c):
        # [L, D, N] -> [L, 128, c, N] with row index = 128*ci + p
        L_, D_, N_ = w.shape
        return np.ascontiguousarray(
            w.reshape(L_, c, 128, N_).transpose(0, 2, 1, 3)).astype(BF16)

    in_maps = []
    for c in range(NCORES):
        b, h = c // 2, c % 2
        nds = slice(NDH * h, NDH * h + NDH)
        dis = slice(DIH * h, DIH * h + DIH)
        rkTh = np.ascontiguousarray(
            rk[:, :, nds].transpose(0, 2, 1).reshape(L, 4, 128, KLEN)
        ).astype(BF16)
        memTb = np.ascontiguousarray(memory[:, b].transpose(0, 2, 1))  # [L,1024,512]
        embTh = embT[:, VSH * h : VSH * h + VSH]                       # [1024, VSH]
        embT4 = np.ascontiguousarray(
            embTh.reshape(8, 128, NVT, VT).transpose(2, 1, 0, 3))      # [NVT,128,8,VT]
        x0 = emb[np.asarray(data[b])].astype(F32)                      # [512, 1024]
        in_maps.append({
            "x0": np.ascontiguousarray(x0.reshape(4, 128, D_MODEL).transpose(1, 0, 2)),
            "memT": chunk(memTb, 8),
            "wq": chunk(Wq[:, :, nds], 8),
            "wk": chunk(Wkv[:, :, nds], 8),
            "wv": chunk(Wkv[:, :, D_MODEL + NDH * h : D_MODEL + NDH * h + NDH], 8),
            "rkT": rkTh,
            "wo": chunk(Wo[:, nds, :], 4),
            "w1": chunk(ffW1[:, :, dis], 8),
            "w2": chunk(ffW2[:, dis, :], 16),
            "b1": np.ascontiguousarray(
                ffb1[:, dis].reshape(L, 16, 128).transpose(0, 2, 1)).astype(F32),
            "b2": (np.asarray(ffb2) * 0.5).astype(F32),
            "g1": np.asarray(ln1_g).astype(F32),
            "bg1": np.asarray(ln1_b).astype(F32),
            "g2": np.asarray(ln2_g).astype(F32),
            "bg2": np.asarray(ln2_b).astype(F32),
            "bw": np.ascontiguousarray(bwf[nds].reshape(4, 128).T),
            "br": np.ascontiguousarray(brf[nds].reshape(4, 128).T),
            "embT": embT4,
        })
    return in_maps


def _combine(results, target, emb):
    nll = np.zeros((BSZ, QLEN), dtype=np.float64)
    for b in range(BSZ):
        r0, r1 = results[2 * b], results[2 * b + 1]
        lm = np.concatenate([r0["lmax"], r1["lmax"]], axis=-1).astype(np.float64)
        ls = np.concatenate([r0["lsum"], r1["lsum"]], axis=-1).astype(np.float64)
        M = lm.max(-1)                                   # [128, 4]
        Z = (ls * np.exp(lm - M[..., None])).sum(-1)     # [128, 4]
        logZ = (M + np.log(Z)).transpose(1, 0).reshape(QLEN)  # i = 128*qc + p
        xf = r0["xout"].astype(BF16).astype(np.float64)
        et = emb[np.asarray(target[b])].astype(BF16).astype(np.float64)
        tgt = (xf * et).sum(-1)
        nll[b] = logZ - tgt
    return nll.astype(F32).reshape(-1).reshape(QLEN, BSZ)


def kernel(**inputs):
    nc = _get_nc()
    data = np.asarray(inputs["data"])
    target = np.asarray(inputs["target"])
    emb = np.asarray(inputs["emb"], dtype=F32)
    in_maps = _prep_inputs(
        data, np.asarray(inputs["memory"], dtype=F32), emb,
        np.asarray(inputs["Wq"], dtype=F32), np.asarray(inputs["Wkv"], dtype=F32),
        np.asarray(inputs["Wr"], dtype=F32), np.asarray(inputs["Wo"], dtype=F32),
        np.asarray(inputs["ffW1"], dtype=F32), np.asarray(inputs["ffb1"], dtype=F32),
        np.asarray(inputs["ffW2"], dtype=F32), np.asarray(inputs["ffb2"], dtype=F32),
        np.asarray(inputs["ln1_g"], dtype=F32), np.asarray(inputs["ln1_b"], dtype=F32),
        np.asarray(inputs["ln2_g"], dtype=F32), np.asarray(inputs["ln2_b"], dtype=F32),
        np.asarray(inputs["bias_w"], dtype=F32), np.asarray(inputs["bias_r"], dtype=F32),
    )
    res = run_bass_kernel_spmd(nc, in_maps, core_ids=list(range(NCORES)))
    return _combine(res.results, target, emb)



# revision 39
# speedup vs baseline: 1.0491x; 1.0491x over previous
"""MemTransformerLM (Transformer-XL) forward pass on 8 TRN2 NeuronCores.

Sharding: core c handles batch b = c//2 and tensor-parallel half h = c%2
(heads 8h..8h+8 of 16; FFN inner columns 2048h..2048h+2048 of 4096).
Pairwise AllReduce (cores 2b, 2b+1) after the attention output projection and
after FFN. Vocab for the final logsumexp is split 16000 per core in the pair;
host combines per-tile (max, sumexp) partials and computes the NLL.

All matmuls run in bf16 with fp32 PSUM accumulation; the residual stream,
layernorm statistics, and softmax denominators stay fp32.

Attention works on TRANSPOSED scores (scoreT[j, i], keys on partitions) so
the softmax'd probs feed the PV matmul straight from SBUF with no transpose:
  AC^T:  matmul(lhsT=kT[64d, j-chunk], rhs=qbwT[64d, i])       -> [j, i] psum
  BD^T:  added into the same psum via an identity matmul from SBUF.
  exp:   ACT psum -> SBUF bf16 probT (no max-subtraction: |s*scale| < ~8).
  PV:    matmul(lhsT=vv[j, 65-wide head block], rhs=probT) accumulated over
         j-chunks; the 65th lhsT column is all-ones, so psum row 64 is the
         softmax denominator for free. A reciprocal + partition-broadcast +
         multiply normalizes on the way out to pvT_all.

rel_shift+mask trick: BD[i, j] = pre[i, r=j-i+qlen-1] where pre[i, r] =
(q_i+br)@rk_r. pre rows are written to a DRAM buffer with row stride W=1536
(cols 0..1023 = pre, cols 1024..1535 pre-filled with -1e30 once). Reading
flat[511 + 1535*i + j] gives the shifted matrix; masked positions (j > i+512
<=> r > 1023) land exactly in the -1e30 pad, so no mask op is needed at all.
One dma_start_transpose with that strided source AP yields bdT[j, i] directly.
"""

import numpy as np
import ml_dtypes

import concourse.bass as bass
import concourse.mybir as mybir
import concourse.tile as tile
from concourse import bacc
from concourse.bass_utils import run_bass_kernel_spmd

# Model dims (hardcoded per problem spec)
L = 6
D_MODEL = 1024
D_HEAD = 64
D_INNER = 4096
BSZ = 4
QLEN = 512
MLEN = 512
KLEN = MLEN + QLEN
VOCAB = 32000
SCALE = 1.0 / (D_HEAD ** 0.5)
EPS = 1e-5

NCORES = 8
NDH = 512          # nd per core (8 heads x 64)
DIH = 2048         # ffn inner per core
VSH = VOCAB // 2   # vocab per core (split across the pair)
VT = 500           # vocab tile width
NVT = VSH // VT    # 32
PREW = KLEN + 512  # pre-buffer row width (cols KLEN.. = -1e30 mask pad)
SCR_BUFS = 3       # dram pool slots for the pre buffer

DT = mybir.dt.float32
BF = mybir.dt.bfloat16
F32 = np.float32
BF16 = ml_dtypes.bfloat16

PAIRS = [[0, 1], [2, 3], [4, 5], [6, 7]]

DEBUG_DUMP = False  # add bdT/probT/pv debug outputs for layer 0, head 0

_CACHE: dict = {}


def _build():
    nc = bacc.Bacc("TRN2", target_bir_lowering=False, debug=False, num_devices=NCORES)

    # ---- I/O ----
    x0_in = nc.dram_tensor("x0", [128, 4, D_MODEL], DT, kind="ExternalInput")
    memT_in = nc.dram_tensor("memT", [L, 128, 8, MLEN], BF, kind="ExternalInput")
    wq_in = nc.dram_tensor("wq", [L, 128, 8, NDH], BF, kind="ExternalInput")
    wk_in = nc.dram_tensor("wk", [L, 128, 8, NDH], BF, kind="ExternalInput")
    wv_in = nc.dram_tensor("wv", [L, 128, 8, NDH], BF, kind="ExternalInput")
    rkT_in = nc.dram_tensor("rkT", [L, 4, 128, KLEN], BF, kind="ExternalInput")
    wo_in = nc.dram_tensor("wo", [L, 128, 4, D_MODEL], BF, kind="ExternalInput")
    w1_in = nc.dram_tensor("w1", [L, 128, 8, DIH], BF, kind="ExternalInput")
    w2_in = nc.dram_tensor("w2", [L, 128, 16, D_MODEL], BF, kind="ExternalInput")
    b1_in = nc.dram_tensor("b1", [L, 128, 16], DT, kind="ExternalInput")
    b2_in = nc.dram_tensor("b2", [L, D_MODEL], DT, kind="ExternalInput")
    g1_in = nc.dram_tensor("g1", [L, D_MODEL], DT, kind="ExternalInput")
    bg1_in = nc.dram_tensor("bg1", [L, D_MODEL], DT, kind="ExternalInput")
    g2_in = nc.dram_tensor("g2", [L, D_MODEL], DT, kind="ExternalInput")
    bg2_in = nc.dram_tensor("bg2", [L, D_MODEL], DT, kind="ExternalInput")
    bw_in = nc.dram_tensor("bw", [128, 4], DT, kind="ExternalInput")
    br_in = nc.dram_tensor("br", [128, 4], DT, kind="ExternalInput")
    embT_in = nc.dram_tensor("embT", [NVT, 128, 8, VT], BF, kind="ExternalInput")

    xout = nc.dram_tensor("xout", [QLEN, D_MODEL], DT, kind="ExternalOutput")
    lmax_out = nc.dram_tensor("lmax", [128, 4, NVT], DT, kind="ExternalOutput")
    lsum_out = nc.dram_tensor("lsum", [128, 4, NVT], DT, kind="ExternalOutput")
    if DEBUG_DUMP:
        dbg_bdT = nc.dram_tensor("dbg_bdT", [128, 8, QLEN], BF, kind="ExternalOutput")
        dbg_prT = nc.dram_tensor("dbg_prT", [128, 8, QLEN], BF, kind="ExternalOutput")
        dbg_pv = nc.dram_tensor("dbg_pv", [64, QLEN], BF, kind="ExternalOutput")
        dbg_den = nc.dram_tensor("dbg_den", [1, QLEN], DT, kind="ExternalOutput")
        dbg_recb = nc.dram_tensor("dbg_recb", [64, QLEN], DT, kind="ExternalOutput")
        dbg_vv = nc.dram_tensor("dbg_vv", [128, 65], BF, kind="ExternalOutput")

    with tile.TileContext(nc) as tc:
        with (
            tc.tile_pool(name="const", bufs=1) as constp,
            tc.tile_pool(name="res", bufs=1) as resp,
            tc.tile_pool(name="wts", bufs=1) as wtp,
            tc.tile_pool(name="act", bufs=1) as actp,
            tc.tile_pool(name="xt2", bufs=2) as xtp,
            tc.tile_pool(name="ncc", bufs=2) as nccp,
            tc.tile_pool(name="arp", bufs=1) as arp,
            tc.tile_pool(name="tr", bufs=2) as trp,
            tc.tile_pool(name="bdp", bufs=2) as bdp,
            tc.tile_pool(name="pr2", bufs=1) as prp,
            tc.tile_pool(name="rcp", bufs=1) as rcp,
            tc.tile_pool(name="small", bufs=2) as smp,
            tc.tile_pool(name="ps_sc", bufs=2, space="PSUM") as psS,
            tc.tile_pool(name="ps_proj", bufs=2, space="PSUM") as psP,
            tc.tile_pool(name="ps_pv", bufs=2, space="PSUM") as psV,
            tc.tile_pool(name="dram", bufs=3, space="DRAM") as dramp,
        ):
            psO = psP  # out-proj / FFN2 share the projection psum slots
            bw_t = constp.tile([128, 4], DT)
            br_t = constp.tile([128, 4], DT)
            nc.sync.dma_start(bw_t[:], bw_in[:])
            nc.sync.dma_start(br_t[:], br_in[:])

            # 128x128 bf16 identity (for the BD += identity @ bdT psum adds)
            ident = constp.tile([128, 128], BF)
            nc.vector.memset(ident[:], 1.0)
            nc.gpsimd.affine_select(
                out=ident[:], in_=ident[:], pattern=[[-1, 128]],
                compare_op=mybir.AluOpType.is_equal, fill=0.0,
                base=0, channel_multiplier=1,
            )

            # -1e30 pad source for the pre-buffer mask columns
            padsb = constp.tile([128, 512], BF)
            nc.vector.memset(padsb[:], -1e30)

            # residual stream, fp32, one tile per 128-query chunk so the four
            # AG-chunk -> residual -> layernorm chains run independently
            x_res = [resp.tile([128, D_MODEL], DT, name=f"xres{qc}",
                               tag=f"xres{qc}")
                     for qc in range(4)]
            for qc in range(4):
                nc.sync.dma_start(x_res[qc][:], x0_in[:, qc, :])
            lmax_sb = resp.tile([128, 4, NVT], DT)
            lsum_sb = resp.tile([128, 4, NVT], DT)

            # persistent pre buffers (explicit round-robin across heads);
            # mask pad columns KLEN..PREW filled with -1e30 exactly once
            scr_bufs = []
            for i in range(SCR_BUFS):
                scr = dramp.tile([QLEN * PREW], BF, tag=f"bdsc{i}")
                scr2d = scr.rearrange("(q w) -> q w", w=PREW)
                for qc in range(4):
                    nc.sync.dma_start(
                        scr2d[128 * qc : 128 * qc + 128, KLEN:PREW], padsb[:]
                    )
                scr_bufs.append((scr, scr2d))
            scr_rr = [0]  # round-robin cursor

            # persistent PV lhsT tile; ones half-columns set once
            vv_all = actp.tile([128, 8, 8 * 2 * D_HEAD], BF, tag="vvf")
            vv_ones = vv_all.rearrange("p k (h w) -> p k h w", w=2 * D_HEAD)
            nc.vector.memset(vv_ones[:, :, :, D_HEAD : 2 * D_HEAD], 1.0)

            def transpose_x():
                """Transpose x_res into [128, 8(dc), QLEN] bf16 via cast-DMA to DRAM
                plus two XBAR transpose-DMAs (chunk layout: row = 128*dc + p)."""
                xsc = dramp.tile([QLEN, D_MODEL], BF, tag="xsc")
                x2d = xsc.rearrange("(c p) d -> p c d", p=128)
                for qc in range(4):
                    nc.gpsimd.dma_start(x2d[:, qc, :], x_res[qc][:])
                dest = xtp.tile([128, 8, QLEN], BF, tag="xt")
                nc.sync.dma_start_transpose(dest[:, 0:4, :], xsc[:, 0 : D_MODEL // 2])
                nc.sync.dma_start_transpose(dest[:, 4:8, :], xsc[:, D_MODEL // 2 :])
                return dest

            for l in range(L):
                # ---- weight loads (wq aliases wo's slot: disjoint lifetimes) ----
                wq_t = wtp.tile([128, 8, NDH], BF, tag="wqo")
                wk_t = wtp.tile([128, 8, NDH], BF, tag="wk")
                wv_t = wtp.tile([128, 8, NDH], BF, tag="wv")
                w1_t = wtp.tile([128, 8, DIH], BF, tag="wff")
                nc.sync.dma_start(wq_t[:], wq_in[l])
                nc.sync.dma_start(wk_t[:], wk_in[l])
                nc.sync.dma_start(wv_t[:], wv_in[l])
                nc.sync.dma_start(w1_t[:], w1_in[l])
                b1_t = wtp.tile([128, 16], DT, tag="bb")
                nc.sync.dma_start(b1_t[:], b1_in[l])

                memT_t = actp.tile([128, 8, MLEN], BF, tag="memT")
                nc.sync.dma_start(memT_t[:], memT_in[l])
                xT_t = transpose_x()

                # ---- attention ----
                # v (all heads) as PV lhsT blocks of 128: cols [128h,128h+64)
                # = v, cols [128h+64,128h+128) = 1.0, so PV psum rows 64:128
                # hold the softmax denominator replicated 64x (lane-aligned
                # for the reciprocal + normalize multiply).
                vv4 = vv_all.rearrange("p k (h w) -> p k h w", w=2 * D_HEAD)
                for kc in range(8):
                    vps = psP.tile([128, QLEN], DT, tag="proj")
                    src = memT_t if kc < 4 else xT_t
                    ksl = slice(128 * (kc % 4), 128 * (kc % 4) + 128)
                    for dc in range(8):
                        nc.tensor.matmul(
                            vps[:], src[:, dc, ksl], wv_t[:, dc, :],
                            start=(dc == 0), stop=(dc == 7),
                        )
                    vps8 = vps.rearrange("p (h w) -> p h w", w=D_HEAD)
                    if kc % 2 == 0:
                        nc.scalar.copy(vv4[:, kc, :, 0:D_HEAD], vps8[:])
                    else:
                        nc.vector.tensor_copy(vv4[:, kc, :, 0:D_HEAD], vps8[:])
                vv = vv_all
                pvT_all = actp.tile([128, 4, QLEN], BF, tag="pvT")

                def ncc_proj(ncc):
                    nsl = slice(128 * ncc, 128 * ncc + 128)
                    # q^T (+bw / +br) for this ncc chunk
                    qps = psP.tile([128, QLEN], DT, tag="proj")
                    for dc in range(8):
                        nc.tensor.matmul(
                            qps[:], wq_t[:, dc, nsl], xT_t[:, dc, :],
                            start=(dc == 0), stop=(dc == 7),
                        )
                    qbwT = nccp.tile([128, QLEN], BF, tag="qbw")
                    qbrT = nccp.tile([128, QLEN], BF, tag="qbr")
                    nc.scalar.add(qbwT[:], qps[:], bw_t[:, ncc : ncc + 1])
                    nc.scalar.add(qbrT[:], qps[:], br_t[:, ncc : ncc + 1])
                    # k^T for this ncc chunk
                    kT = nccp.tile([128, KLEN], BF, tag="kT")
                    for kh in range(2):
                        kps = psP.tile([128, QLEN], DT, tag="proj")
                        src = memT_t if kh == 0 else xT_t
                        for dc in range(8):
                            nc.tensor.matmul(
                                kps[:], wk_t[:, dc, nsl], src[:, dc, :],
                                start=(dc == 0), stop=(dc == 7),
                            )
                        nc.vector.tensor_copy(kT[:, 512 * kh : 512 * kh + 512], kps[:])
                    # rk^T for this ncc chunk (host-computed rk = pos_emb @ Wr)
                    rkT = nccp.tile([128, KLEN], BF, tag="rkT")
                    nc.sync.dma_start(rkT[:], rkT_in[l, ncc])
                    return qbwT, qbrT, kT, rkT

                def pre_phase(ncc, hh, qbwT, qbrT, kT, rkT):
                    base = 64 * hh
                    # pre buffer: rows of width W=1536; cols 1024: hold -1e30
                    scr, scr2d = scr_bufs[scr_rr[0] % SCR_BUFS]
                    scr_rr[0] += 1
                    # pre = (q+br)^T-chunk @ rkT, written to DRAM
                    for qc in range(4):
                        pre_sb = trp.tile([128, KLEN], BF, tag="pre_sb")
                        for kh in range(2):
                            pre = psP.tile([128, 512], DT, tag="proj")
                            nc.tensor.matmul(
                                pre[:],
                                qbrT[base : base + 64, 128 * qc : 128 * qc + 128],
                                rkT[base : base + 64, 512 * kh : 512 * kh + 512],
                                start=True, stop=True,
                            )
                            if kh == 0:
                                nc.vector.tensor_copy(
                                    pre_sb[:, 512 * kh : 512 * kh + 512], pre[:]
                                )
                            else:
                                nc.scalar.copy(
                                    pre_sb[:, 512 * kh : 512 * kh + 512], pre[:]
                                )
                        nc.sync.dma_start(
                            scr2d[128 * qc : 128 * qc + 128, 0:KLEN], pre_sb[:]
                        )
                    # shifted + transposed reload: bdT[128p, kc, i] = BD^T[j, i]
                    # with j = 128*kc + p; masked j land in the -1e30 pad.
                    bdT = bdp.tile([128, 8, QLEN], BF, tag="bdT")
                    shifted = bass.AP(
                        scr.tensor,
                        scr.offset + (QLEN - 1),
                        [[PREW - 1, QLEN], [1, KLEN]],
                    )
                    nc.scalar.dma_start_transpose(bdT[:], shifted)
                    return bdT

                def score_phase(ncc, hh, qbwT, kT, bdT):
                    base = 64 * hh
                    h2 = 2 * ncc + hh
                    # scores (transposed) + exp over pairs of 128-key chunks
                    probT = prp.tile([128, 8, QLEN], BF, tag=f"probT{hh}")
                    for kp in range(4):
                        sc = psS.tile([128, 2, 512], DT, tag="sc")
                        for k2 in range(2):
                            kc = 2 * kp + k2
                            nc.tensor.matmul(
                                sc[:, k2, :], ident[:], bdT[:, kc, :],
                                start=True, stop=False,
                            )
                            nc.tensor.matmul(
                                sc[:, k2, :],
                                kT[base : base + 64, 128 * kc : 128 * kc + 128],
                                qbwT[base : base + 64, :],
                                start=False, stop=True,
                            )
                        nc.scalar.activation(
                            probT[:, 2 * kp : 2 * kp + 2, :], sc[:],
                            mybir.ActivationFunctionType.Exp, scale=SCALE,
                        )
                    # PV; psum rows 64:128 = denominator (replicated 64x)
                    pv = psV.tile([128, QLEN], DT, tag="pv")
                    for kc in range(8):
                        nc.tensor.matmul(
                            pv[:],
                            vv[:, kc, 128 * h2 : 128 * h2 + 128],
                            probT[:, kc, :],
                            start=(kc == 0), stop=(kc == 7),
                        )
                    rec = rcp.tile([128, QLEN], DT, tag="rec")
                    nc.vector.reciprocal(rec[64:128, :], pv[64:128, :])
                    nc.vector.tensor_tensor(
                        pvT_all[base : base + 64, ncc, :],
                        pv[0:64, :], rec[64:128, :], mybir.AluOpType.mult,
                    )
                    if DEBUG_DUMP and l == 0 and ncc == 0 and hh == 0:
                        nc.sync.dma_start(dbg_bdT[:], bdT[:])
                        nc.sync.dma_start(dbg_prT[:], probT[:])
                        nc.sync.dma_start(dbg_den[:], rec[64:65, :])
                        nc.sync.dma_start(dbg_recb[:], rec[64:128, :])
                        nc.sync.dma_start(dbg_vv[:], vv[:, 0, 0:65])
                        nc.sync.dma_start(
                            dbg_pv[:], pvT_all[base : base + 64, ncc, :]
                        )

                # software-pipelined head loop: head i+1's pre matmuls fill
                # head i's DMA round-trip (pre write + shifted transpose read)
                nccs = {}
                prev = None
                for idx in range(8):
                    ncc, hh = divmod(idx, 2)
                    if hh == 0:
                        nccs[ncc] = ncc_proj(ncc)
                    qbwT, qbrT, kT, rkT = nccs[ncc]
                    bdT = pre_phase(ncc, hh, qbwT, qbrT, kT, rkT)
                    if prev is not None:
                        score_phase(*prev)
                    prev = (ncc, hh, qbwT, kT, bdT)
                score_phase(*prev)

                # layernorm params (bf16, broadcast to all partitions);
                # parked in a bdT slot (dead once attention scores are done)
                lnb = bdp.tile([128, 8, QLEN], BF, tag="bdT")
                lnb = lnb.rearrange("p h q -> p (h q)")[:, 0 : 4 * D_MODEL]
                lnb = lnb.rearrange("p (i d) -> p i d", d=D_MODEL)
                for i, src in enumerate((g1_in, bg1_in, g2_in, bg2_in)):
                    lnrow = smp.tile([1, D_MODEL], BF, tag="lnrow")
                    nc.gpsimd.dma_start(lnrow[:], src[l : l + 1, :])
                    nc.gpsimd.partition_broadcast(lnb[:, i, :], lnrow[:])

                def layer_norm_qc(goff, qc):
                    xr = x_res[qc][:]
                    # one packed stats tile per chain:
                    # [0:12]=bn_stats (2 groups x 6), [12]=mu, [13]=var,
                    # [14]=std, [15]=rstd
                    st = smp.tile([128, 16], DT, tag=f"lnst{qc}")
                    nc.vector.bn_stats(st[:, 0:6], x_res[qc][:, 0:512])
                    nc.vector.bn_stats(st[:, 6:12], x_res[qc][:, 512:1024])
                    nc.vector.bn_aggr(
                        st[:, 12:14], st[:, 0:12].rearrange("p (n s) -> p n s", s=6)
                    )
                    nc.vector.tensor_scalar_add(st[:, 14:15], st[:, 13:14], EPS)
                    nc.scalar.sqrt(st[:, 14:15], st[:, 14:15])
                    nc.vector.reciprocal(st[:, 15:16], st[:, 14:15])
                    # x = (x - mu) * rstd, one fused pass
                    nc.vector.tensor_scalar(
                        xr, xr, st[:, 12:13], st[:, 15:16],
                        mybir.AluOpType.subtract, mybir.AluOpType.mult,
                    )
                    nc.vector.tensor_tensor(
                        xr, xr, lnb[:, 2 * goff, :], mybir.AluOpType.mult
                    )
                    nc.vector.tensor_tensor(
                        xr, xr, lnb[:, 2 * goff + 1, :], mybir.AluOpType.add
                    )

                # ---- attention out projection + chunked pairwise AllGather;
                # residual add + ln1 pipelined per 128-query chunk ----
                wo_t = wtp.tile([128, 4, D_MODEL], BF, tag="wqo")
                nc.sync.dma_start(wo_t[:], wo_in[l])
                ar_in = dramp.tile([QLEN, D_MODEL], BF, tag="arin")
                ar_out = dramp.tile([4, 2, 128, D_MODEL], BF, tag="arout")
                asb = arp.tile([128, 4, D_MODEL], BF, tag="ar_sb")
                for qc in range(4):
                    for kh in range(2):
                        ops = psO.tile([128, 512], DT, tag="proj")
                        for ncc in range(4):
                            nc.tensor.matmul(
                                ops[:],
                                pvT_all[:, ncc, 128 * qc : 128 * qc + 128],
                                wo_t[:, ncc, 512 * kh : 512 * kh + 512],
                                start=(ncc == 0), stop=(ncc == 3),
                            )
                        nc.vector.tensor_copy(
                            asb[:, qc, 512 * kh : 512 * kh + 512], ops[:]
                        )
                    nc.sync.dma_start(
                        ar_in[128 * qc : 128 * qc + 128, :], asb[:, qc, :]
                    )
                    nc.gpsimd.collective_compute(
                        "AllGather", mybir.AluOpType.bypass,
                        replica_groups=PAIRS,
                        ins=[ar_in[128 * qc : 128 * qc + 128, :].opt()],
                        outs=[ar_out[qc].opt()],
                    )
                for qc in range(4):
                    # both pair partials (own + partner) summed into asb chunk
                    nc.sync.dma_start(asb[:, qc, :], ar_out[qc, 0])
                    nc.gpsimd.dma_start(
                        asb[:, qc, :], ar_out[qc, 1], accum_op=mybir.AluOpType.add
                    )
                    nc.vector.tensor_tensor(
                        x_res[qc][:], x_res[qc][:], asb[:, qc, :],
                        mybir.AluOpType.add,
                    )
                    layer_norm_qc(0, qc)

                # ---- FFN ----
                xfT = transpose_x()
                hT = actp.tile([128, 16, QLEN], BF, tag="hT")
                for ic in range(16):
                    ps = psP.tile([128, QLEN], DT, tag="proj")
                    for dc in range(8):
                        nc.tensor.matmul(
                            ps[:], w1_t[:, dc, 128 * ic : 128 * ic + 128],
                            xfT[:, dc, :],
                            start=(dc == 0), stop=(dc == 7),
                        )
                    nc.scalar.activation(
                        hT[:, ic, :], ps[:], mybir.ActivationFunctionType.Relu,
                        bias=b1_t[:, ic : ic + 1],
                    )
                w2_t = wtp.tile([128, 16, D_MODEL], BF, tag="wff")
                nc.sync.dma_start(w2_t[:], w2_in[l])
                # b2/2 broadcast (host halves it; both pair partials carry it)
                b2b = wtp.tile([128, D_MODEL], BF, tag="bb")
                b2row = smp.tile([1, D_MODEL], BF, tag="b2row")
                nc.gpsimd.dma_start(b2row[:], b2_in[l : l + 1, :])
                nc.gpsimd.partition_broadcast(b2b[:], b2row[:])
                ar_in2 = dramp.tile([QLEN, D_MODEL], BF, tag="arin")
                ar_out2 = dramp.tile([4, 2, 128, D_MODEL], BF, tag="arout")
                asb2 = arp.tile([128, 4, D_MODEL], BF, tag="ar_sb")
                for qc in range(4):
                    for kh in range(2):
                        ops = psO.tile([128, 512], DT, tag="proj")
                        for ic in range(16):
                            nc.tensor.matmul(
                                ops[:],
                                hT[:, ic, 128 * qc : 128 * qc + 128],
                                w2_t[:, ic, 512 * kh : 512 * kh + 512],
                                start=(ic == 0), stop=(ic == 15),
                            )
                        nc.vector.tensor_tensor(
                            asb2[:, qc, 512 * kh : 512 * kh + 512], ops[:],
                            b2b[:, 512 * kh : 512 * kh + 512],
                            mybir.AluOpType.add,
                        )
                    nc.sync.dma_start(
                        ar_in2[128 * qc : 128 * qc + 128, :], asb2[:, qc, :]
                    )
                    nc.gpsimd.collective_compute(
                        "AllGather", mybir.AluOpType.bypass,
                        replica_groups=PAIRS,
                        ins=[ar_in2[128 * qc : 128 * qc + 128, :].opt()],
                        outs=[ar_out2[qc].opt()],
                    )
                for qc in range(4):
                    nc.sync.dma_start(asb2[:, qc, :], ar_out2[qc, 0])
                    nc.gpsimd.dma_start(
                        asb2[:, qc, :], ar_out2[qc, 1],
                        accum_op=mybir.AluOpType.add,
                    )
                    nc.vector.tensor_tensor(
                        x_res[qc][:], x_res[qc][:], asb2[:, qc, :],
                        mybir.AluOpType.add,
                    )
                    layer_norm_qc(1, qc)

            # ---- final hidden out + unembed partials ----
            # |logits| < ~10 here, so exp needs no max-subtraction; lmax
            # output stays 0 and the host combine handles it unchanged.
            xo4 = xout.rearrange("(c p) d -> p c d", p=128)
            for qc in range(4):
                nc.sync.dma_start(xo4[:, qc, :], x_res[qc][:])
            nc.vector.memset(lmax_sb[:], 0.0)
            uf = transpose_x()
            for vt in range(NVT):
                # double-buffer embT tiles by ping-ponging two dead weight slots
                et = wtp.tile([128, 8, VT], BF, tag=("wk" if vt % 2 == 0 else "wv"))
                nc.sync.dma_start(et[:], embT_in[vt])
                for qc in range(4):
                    lps = psP.tile([128, QLEN], DT, tag="proj")
                    for dc in range(8):
                        nc.tensor.matmul(
                            lps[:, 0:VT],
                            uf[:, dc, 128 * qc : 128 * qc + 128],
                            et[:, dc, :],
                            start=(dc == 0), stop=(dc == 7),
                        )
                    lsc = trp.tile([128, VT], BF, tag="lsc")
                    nc.scalar.activation(
                        lsc[:], lps[:, 0:VT], mybir.ActivationFunctionType.Exp,
                        accum_out=lsum_sb[:, qc, vt : vt + 1],
                    )
            nc.sync.dma_start(lmax_out[:], lmax_sb[:])
            nc.sync.dma_start(lsum_out[:], lsum_sb[:])

    nc.compile()
    return nc


def _get_nc():
    if "nc" not in _CACHE:
        _CACHE["nc"] = _build()
    return _CACHE["nc"]


def _make_pos():
    pos_seq = np.arange(KLEN - 1, -1, -1, dtype=F32)
    inv_freq = 1.0 / (10000.0 ** (np.arange(0, D_MODEL, 2, dtype=F32) / D_MODEL))
    sin_inp = np.outer(pos_seq, inv_freq).astype(F32)
    return np.concatenate([np.sin(sin_inp), np.cos(sin_inp)], -1).astype(F32)


def _prep_inputs(data, memory, emb, Wq, Wkv, Wr, Wo, ffW1, ffb1, ffW2, ffb2,
                 ln1_g, ln1_b, ln2_g, ln2_b, bias_w, bias_r):
    pos = _make_pos()                                  # [KLEN, D_MODEL]
    rk = np.einsum("kd,ldn->lkn", pos, Wr.astype(F32))  # [L, KLEN, 2*NDH]
    embT = np.ascontiguousarray(emb.T).astype(BF16)    # [D_MODEL, VOCAB]
    bwf = bias_w.reshape(-1).astype(F32)
    brf = bias_r.reshape(-1).astype(F32)

    def chunk(w, c):
        # [L, D, N] -> [L, 128, c, N] with row index = 128*ci + p
        L_, D_, N_ = w.shape
        return np.ascontiguousarray(
            w.reshape(L_, c, 128, N_).transpose(0, 2, 1, 3)).astype(BF16)

    in_maps = []
    for c in range(NCORES):
        b, h = c // 2, c % 2
        nds = slice(NDH * h, NDH * h + NDH)
        dis = slice(DIH * h, DIH * h + DIH)
        rkTh = np.ascontiguousarray(
            rk[:, :, nds].transpose(0, 2, 1).reshape(L, 4, 128, KLEN)
        ).astype(BF16)
        memTb = np.ascontiguousarray(memory[:, b].transpose(0, 2, 1))  # [L,1024,512]
        embTh = embT[:, VSH * h : VSH * h + VSH]                       # [1024, VSH]
        embT4 = np.ascontiguousarray(
            embTh.reshape(8, 128, NVT, VT).transpose(2, 1, 0, 3))      # [NVT,128,8,VT]
        x0 = emb[np.asarray(data[b])].astype(F32)                      # [512, 1024]
        in_maps.append({
            "x0": np.ascontiguousarray(x0.reshape(4, 128, D_MODEL).transpose(1, 0, 2)),
            "memT": chunk(memTb, 8),
            "wq": chunk(Wq[:, :, nds], 8),
            "wk": chunk(Wkv[:, :, nds], 8),
            "wv": chunk(Wkv[:, :, D_MODEL + NDH * h : D_MODEL + NDH * h + NDH], 8),
            "rkT": rkTh,
            "wo": chunk(Wo[:, nds, :], 4),
            "w1": chunk(ffW1[:, :, dis], 8),
            "w2": chunk(ffW2[:, dis, :], 16),
            "b1": np.ascontiguousarray(
                ffb1[:, dis].reshape(L, 16, 128).transpose(0, 2, 1)).astype(F32),
            "b2": (np.asarray(ffb2) * 0.5).astype(F32),
            "g1": np.asarray(ln1_g).astype(F32),
            "bg1": np.asarray(ln1_b).astype(F32),
            "g2": np.asarray(ln2_g).astype(F32),
            "bg2": np.asarray(ln2_b).astype(F32),
            "bw": np.ascontiguousarray(bwf[nds].reshape(4, 128).T),
            "br": np.ascontiguousarray(brf[nds].reshape(4, 128).T),
            "embT": embT4,
        })
    return in_maps


def _combine(results, target, emb):
    nll = np.zeros((BSZ, QLEN), dtype=np.float64)
    for b in range(BSZ):
        r0, r1 = results[2 * b], results[2 * b + 1]
        lm = np.concatenate([r0["lmax"], r1["lmax"]], axis=-1).astype(np.float64)
        ls = np.concatenate([r0["lsum"], r1["lsum"]], axis=-1).astype(np.float64)
        M = lm.max(-1)                                   # [128, 4]
        Z = (ls * np.exp(lm - M[..., None])).sum(-1)     # [128, 4]
        logZ = (M + np.log(Z)).transpose(1, 0).reshape(QLEN)  # i = 128*qc + p
        xf = r0["xout"].astype(BF16).astype(np.float64)
        et = emb[np.asarray(target[b])].astype(BF16).astype(np.float64)
        tgt = (xf * et).sum(-1)
        nll[b] = logZ - tgt
    return nll.astype(F32).reshape(-1).reshape(QLEN, BSZ)


def kernel(**inputs):
    nc = _get_nc()
    data = np.asarray(inputs["data"])
    target = np.asarray(inputs["target"])
    emb = np.asarray(inputs["emb"], dtype=F32)
    in_maps = _prep_inputs(
        data, np.asarray(inputs["memory"], dtype=F32), emb,
        np.asarray(inputs["Wq"], dtype=F32), np.asarray(inputs["Wkv"], dtype=F32),
        np.asarray(inputs["Wr"], dtype=F32), np.asarray(inputs["Wo"], dtype=F32),
        np.asarray(inputs["ffW1"], dtype=F32), np.asarray(inputs["ffb1"], dtype=F32),
        np.asarray(inputs["ffW2"], dtype=F32), np.asarray(inputs["ffb2"], dtype=F32),
        np.asarray(inputs["ln1_g"], dtype=F32), np.asarray(inputs["ln1_b"], dtype=F32),
        np.asarray(inputs["ln2_g"], dtype=F32), np.asarray(inputs["ln2_b"], dtype=F32),
        np.asarray(inputs["bias_w"], dtype=F32), np.asarray(inputs["bias_r"], dtype=F32),
    )
    res = run_bass_kernel_spmd(nc, in_maps, core_ids=list(range(NCORES)))
    return _combine(res.results, target, emb)

